# revision 1
# baseline (speedup 1.0000x reference)
"""BEV deformable cross-attention kernel for 8 Trainium2 NeuronCores.

Strategy (per core): data-parallel over (B x K-half): core c handles batch
b = c//2 and modes k in {3*(c%2) .. +3}, i.e. 36 queries, 288 sample points.

Key algebraic move: grid_sample(conv1x1(bev)) == conv1x1(grid_sample(bev)),
so instead of materializing the two full (256,200,200) conv maps we gather
only the 4 bilinear corners of the 288 sample points from a host-transposed
HWC copy of bev_feat (channels contiguous per pixel -> 2KB indirect reads),
interpolate in 256-d, then apply the 1x1 convs to 288 vectors.

Everything else (offset MLP, DAB-style sine embeddings with on-device range
reduction, positional MLPs, 8-key-per-query attention via selection-matrix
matmuls, output projection + residual) runs on-device in fp32, feature-major
(features on partitions, queries/points on the free axis).
"""
import numpy as np

import concourse.bass as bass
import concourse.mybir as mybir
import concourse.tile as tile_mod
from concourse.bass import AP, IndirectOffsetOnAxis

F32 = mybir.dt.float32
F32R = mybir.dt.float32r
I32 = mybir.dt.int32
AF = mybir.ActivationFunctionType
OP = mybir.AluOpType

# problem constants (hardcoded per contract)
K, B, T, DIM = 6, 4, 12, 256
H, W = 200, 200
HALF = 256
G = 8                      # offset groups == sample points per query
NH = 8                     # heads
HD = 32                    # head dim of value part
NQ = 3 * T                 # queries per core = 36
NPT = NQ * G               # points per core = 288
OFFSET_SCALE = 4.0
PIX_SCALE = float(W / 102.4)          # 1.953125
PIX_BIAS = float(W / 2.0 - 0.5)       # 99.5
SCALE = 64 ** -0.5                    # 0.125
TWO_PI = float(2 * np.pi)
RC = float(3 * 2 ** 22)               # 1.5*2^23 rint magic constant
CHUNKS = [(0, 128), (128, 128), (256, 32)]   # point chunks (start, size)

# ---------------------------------------------------------------- blob layout


class Alloc:
    def __init__(self):
        self.pos = 0
        self.slices = {}

    def add(self, name, width):
        self.slices[name] = (self.pos, width)
        self.pos += width

    def __getitem__(self, name):
        return self.slices[name]


WBLOBS = {
    # most-critical first: con_q weights (smallest possible first transfer)
    "A0": [("wconq", 512), ("bconq", 2)],
    # rest of the critical path (fp32)
    "A": [("bdh", 512), ("bo1rep", 1),
          ("wo2top", 2), ("wo2bot", 2), ("bo2", 1), ("sc4pm", 2),
          ("fq2", 128), ("fk5x", 128), ("fk5y", 128), ("ident", 128)],
    # fat matmul weights (float32r)
    "R": [("wk1", 512), ("wk2", 512), ("wcat", 1024),
          ("s0", 8), ("s1", 8), ("e0", 128), ("e1", 128)],
    # the rest (fp32, needed later)
    "B": [("wq1", 512), ("bq1", 2), ("wq2", 512), ("bq2", 2),
          ("bk1", 2), ("bk2", 2), ("wout", 512), ("bout", 2)],
}


def wblob_layout(which):
    a = Alloc()
    for nm, wd in WBLOBS[which]:
        a.add(nm, wd)
    return a


NAME2BLOB = {nm: which for which, items in WBLOBS.items() for nm, _ in items}


def xblob_layout():
    a = Alloc()
    # rpx1/rpy1: rows [rp; ones] for the K=2 qse phase matmul.
    # rpo: rows [rpexp_x; rpexp_y; ones] for the K=5 kse phase matmul rhs.
    # bpm: per-chunk point-major pixel bias [sc*rpx+99.5, -sc*rpy+99.5].
    for nm, wd in [("deT", 72), ("qsT", 72), ("rpx1", 36), ("rpy1", 36),
                   ("rpo", 288), ("bpm", 6)]:
        a.add(nm, wd)
    return a


def pack_wblobs(weights):
    """weights: dict of numpy arrays (original reference layouts)."""
    lays = {w: wblob_layout(w) for w in WBLOBS}
    wbs = {w: np.zeros((128, lays[w].pos), np.float32) for w in WBLOBS}

    def put(name, arr, rows=128, coloff=0):
        lay = lays[NAME2BLOB[name]]; wb = wbs[NAME2BLOB[name]]
        s, _ = lay[name]
        wb[:rows, s + coloff: s + coloff + arr.shape[1]] = arr

    def put_mm(name, w256):  # (256, Mout) -> blocks (kc, mc) of (128, 128)
        lay = lays[NAME2BLOB[name]]; wb = wbs[NAME2BLOB[name]]
        s, _ = lay[name]
        mcs = w256.shape[1] // 128
        for kc in range(2):
            for mc in range(mcs):
                blk = w256[kc * 128:(kc + 1) * 128, mc * 128:(mc + 1) * 128]
                off = (kc * mcs + mc) * 128
                wb[:, s + off: s + off + 128] = blk

    put_mm("wconq", weights["W_con_q"])
    put("bconq", weights["b_con_q"].reshape(2, 128).T)
    # block-diag Wo1 consts: j = cc*2+h2 covers groups (2j, 2j+1)
    s, _ = lays["A"]["bdh"]
    wo1 = weights["Wo1"]  # (32, 64)
    for j in range(4):
        blk = np.zeros((128, 128), np.float32)
        blk[0:32, 0:64] = wo1 if j % 2 == 0 else 0
        if j % 2 == 0:
            blk[0:32, 0:64] = wo1
            blk[32:64, 64:128] = wo1
        else:
            blk[64:96, 0:64] = wo1
            blk[96:128, 64:128] = wo1
        wbs["A"][:, s + j * 128: s + (j + 1) * 128] = blk
    put("bo1rep", np.tile(weights["bo1"], 2)[:, None])
    wo2 = weights["Wo2"]  # (64, 2)
    top = np.zeros((128, 2), np.float32); top[0:64] = wo2
    bot = np.zeros((128, 2), np.float32); bot[64:128] = wo2
    put("wo2top", top); put("wo2bot", bot)
    put("bo2", weights["bo2"][:, None], rows=2)
    put("sc4pm", np.tile(np.array([[4 * PIX_SCALE, -4 * PIX_SCALE]],
                                  np.float32), (128, 1)))
    i64 = np.arange(128) // 2
    freq = (TWO_PI / (10000.0 ** (i64 / 64.0))).astype(np.float32)
    shift = np.where(np.arange(128) % 2 == 1, np.pi / 2, 0.0).astype(np.float32)
    fq2 = np.stack([freq, shift])                      # (2, 128)
    put("fq2", fq2, rows=2)
    fk5x = np.zeros((5, 128), np.float32)
    fk5x[0] = 4 * freq; fk5x[2] = freq; fk5x[4] = shift
    fk5y = np.zeros((5, 128), np.float32)
    fk5y[1] = 4 * freq; fk5y[3] = freq; fk5y[4] = shift
    put("fk5x", fk5x, rows=5)
    put("fk5y", fk5y, rows=5)
    put("ident", np.eye(128, dtype=np.float32))
    put_mm("wq1", weights["Wq1"]); put("bq1", weights["bq1"].reshape(2, 128).T)
    put_mm("wq2", weights["Wq2"]); put("bq2", weights["bq2"].reshape(2, 128).T)
    put_mm("wk1", weights["Wk1"]); put("bk1", weights["bk1"].reshape(2, 128).T)
    put_mm("wk2", weights["Wk2"]); put("bk2", weights["bk2"].reshape(2, 128).T)
    wcat = np.concatenate([weights["W_con_k"], weights["W_v"]], axis=1)  # (256,512)
    put_mm("wcat", wcat)
    put_mm("wout", weights["W_out"])
    put("bout", weights["b_out"].reshape(2, 128).T)
    d = np.arange(128)
    s0 = np.zeros((128, 8), np.float32)
    s0[d, d // 32] = SCALE
    s1 = np.zeros((128, 8), np.float32)
    s1[d, 4 + d // 32] = SCALE
    put("s0", s0); put("s1", s1)
    e0 = np.zeros((8, 128), np.float32)
    e0[d // 32, d] = 1.0
    e1 = np.zeros((8, 128), np.float32)
    e1[4 + d // 32, d] = 1.0
    put("e0", e0, rows=8); put("e1", e1, rows=8)
    return wbs


def pack_xblob(dec_embed, query_scale, ref_points, b, k0):
    """Per-core input blob: 36 queries = modes k0..k0+2, all T."""
    lay = xblob_layout()
    xb = np.zeros((128, lay.pos), np.float32)
    de = dec_embed[k0:k0 + 3, b].reshape(NQ, DIM)       # (36, 256)
    qs = query_scale[k0:k0 + 3, b].reshape(NQ, DIM)
    rp = ref_points[k0:k0 + 3, b].reshape(NQ, 2)

    s, _ = lay["deT"]
    xb[:, s: s + 36] = de.T[:128]
    xb[:, s + 36: s + 72] = de.T[128:]
    s, _ = lay["qsT"]
    xb[:, s: s + 36] = qs.T[:128]
    xb[:, s + 36: s + 72] = qs.T[128:]
    s, _ = lay["rpx1"]
    xb[0, s: s + 36] = rp[:, 0]
    xb[1, s: s + 36] = 1.0
    s, _ = lay["rpy1"]
    xb[0, s: s + 36] = rp[:, 1]
    xb[1, s: s + 36] = 1.0
    s, _ = lay["rpo"]
    xb[0:2, s: s + 288] = np.tile(rp.T, (1, 8))         # g-major: col = g*36+q
    xb[2, s: s + 288] = 1.0
    s, _ = lay["bpm"]
    rpe = np.tile(rp.T, (1, 8))                         # (2, 288)
    bx = PIX_SCALE * rpe[0] + PIX_BIAS
    by = -PIX_SCALE * rpe[1] + PIX_BIAS
    for c, (c0, cn) in enumerate(CHUNKS):
        xb[:cn, s + 2 * c] = bx[c0:c0 + cn]
        xb[:cn, s + 2 * c + 1] = by[c0:c0 + cn]
    return xb


# --------------------------------------------------------------- tile patches

def _split_drain_and_barrier(self, tick_clock, wait_clock):
    nc = self.nc
    drain_inst = nc.sync.drain()
    wait_clock.add_sem_waits(
        drain_inst.ins, tile_mod.ScopedClock({None: tick_clock.global_clock})
    )
    si = drain_inst.ins.sync_info
    waits = list(si.on_wait)
    if len(waits) > 1:
        si.on_wait = waits[:1]
        for i in range(1, len(waits)):
            extra = nc.sync.drain()
            extra.ins.sync_info = type(si)(on_wait=waits[i: i + 1], on_update=[])
    nc.all_engine_barrier()
    assert self.sems is not None
    popped = nc._tile_sem_poison_stack.pop()
    assert popped is self._sem_poison
    nc.clear_and_free_semaphores(list(self.sems.allocated().values()))


def split_multiwaits(nc):
    """walrus codegen supports a single sync-wait per instruction; split."""
    f = nc.m.functions[0]
    for blk in f.blocks:
        todo = [i for i in blk.instructions
                if i.sync_info is not None and len(i.sync_info.on_wait) > 1]
        for inst in todo:
            si = inst.sync_info
            waits = list(si.on_wait)
            nops = []
            for w in waits[:-1]:
                bi = nc.engines[inst.engine].nop(nofuse=True)
                ni = bi.ins
                for b2 in f.blocks:
                    if b2.instructions and b2.instructions[-1] is ni:
                        b2.instructions.pop()
                        break
                ni.sync_info = type(si)(on_wait=[w], on_update=[])
                nops.append(ni)
            si.on_wait = [waits[-1]]
            pos = blk.instructions.index(inst)
            blk.instructions[pos:pos] = nops


_PATCHED = False


def patch_tile():
    global _PATCHED
    if not _PATCHED:
        tile_mod.TileContext._drain_and_barrier = _split_drain_and_barrier
        _PATCHED = True


# ---------------------------------------------------------------- the kernel

def view3(ap, dims):
    """Build a 3D AP view on top of a 2D tile AP: dims = [[step,count],...]
    applied after the partition dim (ap.ap[0] kept)."""
    return AP(ap.tensor, ap.offset, [ap.ap[0]] + dims)


def build_nc(sim_mode=False, debug=False):
    patch_tile()
    nc = bass.Bass("TRN2")
    wlays = {w: wblob_layout(w) for w in WBLOBS}
    xlay = xblob_layout()

    # row-pair interleaved: bev[y*W+x] = [feat(y,x) (256) | feat(y+1,x) (256)]
    bev = nc.dram_tensor("bev", [H * W, 512], F32, kind="ExternalInput")
    wblA0 = nc.dram_tensor("wblA0", [128, wlays["A0"].pos], F32, kind="ExternalInput")
    wblA = nc.dram_tensor("wblA", [128, wlays["A"].pos], F32, kind="ExternalInput")
    wblR = nc.dram_tensor("wblR", [128, wlays["R"].pos], F32R, kind="ExternalInput")
    wblB = nc.dram_tensor("wblB", [128, wlays["B"].pos], F32, kind="ExternalInput")
    xbl = nc.dram_tensor("xbl", [128, xlay.pos], F32, kind="ExternalInput")
    out = nc.dram_tensor("out", [256, NQ], F32, kind="ExternalOutput")

    dbg = {}
    if debug:
        for nm, shp, dt in [
            ("d_pix", [128, 2], F32),
            ("d_idx", [128, 1], I32), ("d_sam0", [128, 256], F32),
            ("d_sim", [8, 288], F32), ("d_at", [8, 288], F32),
            ("d_kse0", [128, 288], F32R), ("d_posk0", [128, 288], F32),
            ("d_conv0", [128, 288], F32), ("d_qse0", [128, 36], F32),
            ("d_cq0", [128, 36], F32), ("d_h", [128, 144], F32),
            ("d_av0", [128, 36], F32), ("d_w40", [128, 4], F32),
        ]:
            dbg[nm] = nc.dram_tensor(nm, shp, dt, kind="ExternalOutput")

    with tile_mod.TileContext(nc) as tc:
        with (
            tc.tile_pool(name="sbuf", bufs=1) as pool,
            tc.tile_pool(name="psum", bufs=1, space="PSUM") as psum,
        ):
            # warm the {erf,tanh} activation table during the weight DMA
            wt = pool.tile([1, 1], F32)
            nc.vector.memset(wt[:], 0.0)
            warm = pool.tile([1, 1], F32)
            nc.scalar.activation(out=warm[:], in_=wt[:],
                                 func=AF.Sigmoid if sim_mode else AF.Gelu,
                                 bias=0.0)

            xb = pool.tile([128, xlay.pos], F32)
            nc.sync.dma_start(out=xb[:], in_=xbl[:])
            wbA0 = pool.tile([128, wlays["A0"].pos], F32)
            nc.sync.dma_start(out=wbA0[:], in_=wblA0[:])
            wbA = pool.tile([128, wlays["A"].pos], F32)
            nc.sync.dma_start(out=wbA[:], in_=wblA[:])
            wbR = pool.tile([128, wlays["R"].pos], F32R)
            nc.sync.dma_start(out=wbR[:], in_=wblR[:])
            wbB = pool.tile([128, wlays["B"].pos], F32)
            nc.sync.dma_start(out=wbB[:], in_=wblB[:])
            wbtiles = {"A0": wbA0, "A": wbA, "R": wbR, "B": wbB}

            def wsl(name, rows=128, off=0, width=None):
                which = NAME2BLOB[name]
                s, wd = wlays[which][name]
                if width is None:
                    width = wd - off
                return wbtiles[which][0:rows, s + off: s + off + width]

            def xsl(name, rows=128, off=0, width=None):
                s, wd = xlay[name]
                if width is None:
                    width = wd - off
                return xb[0:rows, s + off: s + off + width]

            deT = [xsl("deT", off=mc * 36, width=36) for mc in range(2)]
            qsT = [xsl("qsT", off=mc * 36, width=36) for mc in range(2)]

            # ---- 1. con_q = de @ W_con_q + b  (feature-major, 2 chunks)
            cqS = []
            for mc in range(2):
                p = psum.tile([128, 288], F32, space="PSUM", tag="psA", bufs=3, name="cqP")
                for kc in range(2):
                    nc.tensor.matmul(
                        out=p[:, :36], lhsT=wsl("wconq", off=(kc * 2 + mc) * 128, width=128),
                        rhs=deT[kc], start=(kc == 0), stop=(kc == 1))
                t = pool.tile([128, 36], F32, tag=f"cqS{mc}")
                nc.scalar.activation(out=t[:], in_=p[:, :36], func=AF.Identity,
                                     bias=wsl("bconq", off=mc, width=1))
                cqS.append(t)
            if debug:
                nc.sync.dma_start(out=dbg["d_cq0"][:], in_=cqS[0][:])

            # ---- 2. h = gelu(grouped con_q @ Wo1 + bo1): 4 block-diag mms
            hP = psum.tile([128, 288], F32, space="PSUM", tag="psA", bufs=3, name="hP")
            for j in range(4):
                cc = j // 2
                nc.tensor.matmul(
                    out=hP[:, j * 36:(j + 1) * 36],
                    lhsT=wsl("bdh", off=j * 128, width=128),
                    rhs=cqS[cc][:], start=True, stop=True)
            hS = pool.tile([128, 144], F32)
            if sim_mode:
                hx = pool.tile([128, 144], F32)
                nc.scalar.activation(out=hx[:], in_=hP[:, :144], func=AF.Identity,
                                     bias=wsl("bo1rep"))
                he = pool.tile([128, 144], F32)
                nc.scalar.activation(out=he[:], in_=hx[:], func=AF.Sigmoid,
                                     scale=float(1 / np.sqrt(2)), bias=0.0)
                nc.vector.tensor_scalar(out=he[:], in0=he[:], scalar1=0.5,
                                        scalar2=0.5, op0=OP.mult, op1=OP.add)
                nc.vector.tensor_tensor(out=hS[:], in0=hx[:], in1=he[:], op=OP.mult)
            else:
                # HW act table 'gelu' is the exact erf-based gelu
                nc.scalar.activation(out=hS[:], in_=hP[:, :144], func=AF.Gelu,
                                     bias=wsl("bo1rep"))
            if debug:
                nc.sync.dma_start(out=dbg["d_h"][:], in_=hS[:])

            # ---- 3. offsets: 2 matmuls (even/odd groups) into strided psum;
            # tanh lands in rows 0:2 of the kse-rhs tile (rows 2:5 = host
            # [rpexp_x; rpexp_y; ones]); grid math uses the tanh directly.
            offP = psum.tile([2, 288], F32, space="PSUM", tag="psA", bufs=3, name="offP")
            for m, wn in [(0, "wo2top"), (1, "wo2bot")]:
                nc.tensor.matmul(
                    out=offP[:, m * 144:(m + 1) * 144],
                    lhsT=wsl(wn, width=2),
                    rhs=hS[:], start=True, stop=True)
            kseRhs = pool.tile([5, 288], F32)
            s_rpo, _ = xlay["rpo"]
            nc.sync.dma_start(out=kseRhs[2:5, :], in_=xbl[0:3, s_rpo:s_rpo + 288])
            # tanh both halves in one op: 4D views interleave g-major cols
            kra = kseRhs[0:2, :]
            opa2 = offP[:]
            nc.scalar.activation(
                out=AP(kra.tensor, kra.offset,
                       [kra.ap[0], [72, 4], [36, 2], [1, 36]]),
                in_=AP(opa2.tensor, opa2.offset,
                       [opa2.ap[0], [36, 4], [144, 2], [1, 36]]),
                func=AF.Tanh, bias=wsl("bo2", rows=2, width=1))

            # ---- 4+5. transpose tanh to point-major, then per-point
            # geometry; gathers are issued per chunk as soon as idx is ready,
            # bilinear weights are built afterwards (off the gather path).
            idxI, w4, frs, pixdbg = [], [], [], None
            gA = []
            s_bpm, _ = xlay["bpm"]
            for c, (c0, cn) in enumerate(CHUNKS):
                tp = psum.tile([128, 2], F32, space="PSUM", tag="psA", bufs=3, name="tpP")
                nc.tensor.transpose(out=tp[:cn, :], in_=kseRhs[0:2, c0:c0 + cn],
                                    identity=wsl("ident", rows=2, width=2))
                pix = pool.tile([128, 2], F32, tag=f"pix{c}", name=f"pix{c}")
                nc.vector.tensor_tensor(out=pix[:cn, :], in0=tp[:cn, :],
                                        in1=wsl("sc4pm", rows=cn, width=2),
                                        op=OP.mult)
                nc.vector.tensor_tensor(
                    out=pix[:cn, :], in0=pix[:cn, :],
                    in1=xb[0:cn, s_bpm + 2 * c: s_bpm + 2 * c + 2], op=OP.add)
                f0 = pool.tile([128, 2], F32, tag=f"f0{c}", name=f"f0{c}")
                nc.vector.tensor_scalar(out=f0[:cn, :], in0=pix[:cn, :],
                                        scalar1=-0.5, scalar2=float(RC),
                                        op0=OP.add, op1=OP.add)
                nc.vector.tensor_scalar(out=f0[:cn, :], in0=f0[:cn, :],
                                        scalar1=float(-RC), scalar2=None,
                                        op0=OP.add)
                fr = pool.tile([128, 2], F32, tag=f"fr{c}", name=f"fr{c}")
                nc.vector.tensor_tensor(out=fr[:cn, :], in0=pix[:cn, :],
                                        in1=f0[:cn, :], op=OP.subtract)
                frs.append(fr)
                idf = pool.tile([128, 1], F32, tag=f"idf{c}", name=f"idf{c}")
                nc.vector.tensor_scalar(out=idf[:cn, :], in0=f0[:cn, 1:2],
                                        scalar1=float(W), scalar2=None,
                                        op0=OP.mult)
                nc.vector.tensor_tensor(out=idf[:cn, :], in0=idf[:cn, :],
                                        in1=f0[:cn, 0:1], op=OP.add)
                ii = pool.tile([128, 1], I32, tag=f"idxI{c}", name=f"idxI{c}")
                nc.vector.tensor_copy(out=ii[:cn, :], in_=idf[:cn, :])
                idxI.append(ii)
                ga = pool.tile([128, 1024], F32, tag=f"gA{c}", name=f"gA{c}")
                nc.gpsimd.indirect_dma_start(
                    out=ga[:cn, :], out_offset=None, in_=bev[:],
                    in_offset=IndirectOffsetOnAxis(ap=ii[:cn, :], axis=0))
                gA.append(ga)
                if debug and c == 0:
                    pixdbg = pix
            # bilinear weights (Pc, 4) = [w00, w10, w01, w11]
            for c, (c0, cn) in enumerate(CHUNKS):
                fr = frs[c]
                wxp = pool.tile([128, 2], F32, tag=f"wxp{c}", name=f"wxp{c}")
                nc.vector.tensor_scalar(out=wxp[:cn, 0:1], in0=fr[:cn, 0:1],
                                        scalar1=-1.0, scalar2=1.0,
                                        op0=OP.mult, op1=OP.add)
                nc.scalar.copy(out=wxp[:cn, 1:2], in_=fr[:cn, 0:1])
                wyp = pool.tile([128, 2], F32, tag=f"wyp{c}", name=f"wyp{c}")
                nc.vector.tensor_scalar(out=wyp[:cn, 0:1], in0=fr[:cn, 1:2],
                                        scalar1=-1.0, scalar2=1.0,
                                        op0=OP.mult, op1=OP.add)
                nc.scalar.copy(out=wyp[:cn, 1:2], in_=fr[:cn, 1:2])
                w4c = pool.tile([128, 4], F32, tag=f"w4{c}", name=f"w4{c}")
                wxa = wxp[:cn, :]
                wya = wyp[:cn, :]
                nc.vector.tensor_tensor(
                    out=view3(w4c[:cn, :], [[2, 2], [1, 2]]),
                    in0=AP(wxa.tensor, wxa.offset, [wxa.ap[0], [0, 2], [1, 2]]),
                    in1=AP(wya.tensor, wya.offset, [wya.ap[0], [1, 2], [0, 2]]),
                    op=OP.mult)
                w4.append(w4c)
            if debug:
                nc.sync.dma_start(out=dbg["d_pix"][:], in_=pixdbg[:])
                nc.sync.dma_start(out=dbg["d_idx"][:], in_=idxI[0][:])
                nc.sync.dma_start(out=dbg["d_w40"][:], in_=w4[0][:])

            # ---- helpers: sine embedding (feature-major halves)
            def sine_embed(lhs_name, lhs_rows, rhs_ap, n, tag, odt=F32):
                """phase = freq (x) meters + shift; one 128-row half."""
                ph = psum.tile([128, 288], F32, space="PSUM", tag="psA", bufs=3, name="phP")
                nc.tensor.matmul(out=ph[:, :n], lhsT=wsl(lhs_name, rows=lhs_rows),
                                 rhs=rhs_ap, start=True, stop=True)
                m1t = pool.tile([128, n], F32, tag=f"sm1{tag}")
                if n > 64:
                    nc.scalar.activation(out=m1t[:], in_=ph[:, :n], func=AF.Copy,
                                         scale=float(1.0 / TWO_PI), bias=float(RC))
                else:
                    nc.vector.tensor_scalar(out=m1t[:], in0=ph[:, :n],
                                            scalar1=float(1.0 / TWO_PI), scalar2=RC,
                                            op0=OP.mult, op1=OP.add)
                k2t = pool.tile([128, n], F32, tag=f"sk2{tag}")
                nc.vector.tensor_scalar(out=k2t[:], in0=m1t[:], scalar1=-RC,
                                        scalar2=-TWO_PI, op0=OP.add, op1=OP.mult)
                yt = pool.tile([128, n], F32, tag=f"sy{tag}")
                nc.vector.tensor_tensor(out=yt[:], in0=ph[:, :n], in1=k2t[:], op=OP.add)
                nc.vector.tensor_scalar(out=yt[:], in0=yt[:],
                                        scalar1=float(np.pi),
                                        scalar2=float(-np.pi),
                                        op0=OP.min, op1=OP.max)
                st = pool.tile([128, n], odt, tag=f"se{tag}")
                nc.scalar.activation(out=st[:], in_=yt[:], func=AF.Sin)
                return st

            def mlp2(inp2, n, wn1, bn1, wn2, bn2, tag, middt=F32):
                """two-layer MLP relu(x@W1+b1)@W2+b2, feature-major chunks."""
                mid = []
                for mc in range(2):
                    p = psum.tile([128, 288], F32, space="PSUM", tag="psA", bufs=3, name="m1P")
                    for kc in range(2):
                        nc.tensor.matmul(
                            out=p[:, :n], lhsT=wsl(wn1, off=(kc * 2 + mc) * 128, width=128),
                            rhs=inp2[kc][:], start=(kc == 0), stop=(kc == 1))
                    t = pool.tile([128, n], middt, tag=f"m1S{tag}{mc}")
                    nc.scalar.activation(out=t[:], in_=p[:, :n], func=AF.Relu,
                                         bias=wsl(bn1, off=mc, width=1))
                    mid.append(t)
                outs = []
                for mc in range(2):
                    p = psum.tile([128, 288], F32, space="PSUM", tag="psA", bufs=3, name="m2P")
                    for kc in range(2):
                        nc.tensor.matmul(
                            out=p[:, :n], lhsT=wsl(wn2, off=(kc * 2 + mc) * 128, width=128),
                            rhs=mid[kc][:], start=(kc == 0), stop=(kc == 1))
                    t = pool.tile([128, n], F32, tag=f"m2S{tag}{mc}")
                    nc.scalar.activation(out=t[:], in_=p[:, :n], func=AF.Identity,
                                         bias=wsl(bn2, off=mc, width=1))
                    outs.append(t)
                return outs

            # ---- 10a. qse sins first: their Sin triggers the trig table
            # load in the ACT-idle window before the kse sins need it.
            qse = [sine_embed("fq2", 2, xsl("rpy1", rows=2), 36, "qy"),
                   sine_embed("fq2", 2, xsl("rpx1", rows=2), 36, "qx")]
            if debug:
                nc.sync.dma_start(out=dbg["d_qse0"][:], in_=qse[0][:])
            # ---- 11. kse + pos_k (288 cols)
            kse = [sine_embed("fk5y", 5, kseRhs[:], 288, "ky", odt=F32R),
                   sine_embed("fk5x", 5, kseRhs[:], 288, "kx", odt=F32R)]
            if debug:
                nc.sync.dma_start(out=dbg["d_kse0"][:], in_=kse[0][:])
            pkS = mlp2(kse, 288, "wk1", "bk1", "wk2", "bk2", "k", middt=F32R)
            if debug:
                nc.sync.dma_start(out=dbg["d_posk0"][:], in_=pkS[0][:])
            # prefetch the exp table: queued on ACT after the Sin burst and
            # pos_k activations, well before the softmax Exp needs it.
            wt2 = pool.tile([1, 1], F32)
            nc.scalar.activation(out=wt2[:], in_=pkS[1][0:1, 0:1], func=AF.Exp)

            # ---- 7. bilinear combine -> sampled (point-major)
            # gather quarters: [c00 | c01 | c10 | c11]; weights cols
            # [w00, w10, w01, w11] -> quarter j uses w4 col [0, 2, 1, 3][j]
            sam = []
            for c, (c0, cn) in enumerate(CHUNKS):
                t1 = pool.tile([128, 256], F32, tag=f"bt1{c}", name=f"bt1{c}")
                t2 = pool.tile([128, 256], F32, tag=f"bt2{c}", name=f"bt2{c}")
                sm = pool.tile([128, 256], F32, tag=f"sam{c}", name=f"sam{c}")
                g = gA[c]
                nc.scalar.activation(out=t1[:cn, :], in_=g[:cn, 0:256],
                                     func=AF.Copy, scale=w4[c][:cn, 0:1])
                nc.vector.tensor_scalar(out=t2[:cn, :], in0=g[:cn, 256:512],
                                        scalar1=w4[c][:cn, 2:3], scalar2=None,
                                        op0=OP.mult)
                nc.vector.tensor_tensor(out=t1[:cn, :], in0=t1[:cn, :],
                                        in1=t2[:cn, :], op=OP.add)
                nc.scalar.activation(out=t2[:cn, :], in_=g[:cn, 512:768],
                                     func=AF.Copy, scale=w4[c][:cn, 1:2])
                nc.vector.tensor_tensor(out=t1[:cn, :], in0=t1[:cn, :],
                                        in1=t2[:cn, :], op=OP.add)
                nc.vector.tensor_scalar(out=t2[:cn, :], in0=g[:cn, 768:1024],
                                        scalar1=w4[c][:cn, 3:4], scalar2=None,
                                        op0=OP.mult)
                nc.vector.tensor_tensor(out=sm[:cn, :], in0=t1[:cn, :],
                                        in1=t2[:cn, :], op=OP.add)
                sam.append(sm)
            if debug:
                nc.sync.dma_start(out=dbg["d_sam0"][:], in_=sam[0][:])

            # ---- 8. transpose sampled to feature-major (256, 288) = 2 tiles
            samT = [pool.tile([128, 288], F32R, tag=f"samT{fc}", name=f"samT{fc}")
                    for fc in range(2)]
            for c, (c0, cn) in enumerate(CHUNKS):
                for fc in range(2):
                    tp = psum.tile([128, 128], F32, space="PSUM", tag="psA", bufs=3, name="samtp")
                    nc.tensor.transpose(out=tp[:, :cn],
                                        in_=sam[c][:cn, fc * 128:(fc + 1) * 128],
                                        identity=wsl("ident", rows=cn, width=cn))
                    nc.scalar.copy(out=samT[fc][:, c0:c0 + cn], in_=tp[:, :cn])

            # ---- 9. conv: con_k / v = sampled @ [W_con_k | W_v], split at the
            # g-block boundary 252 so the [0:252] part (point chunks 0+1 only)
            # runs while chunk 2's gather is still in flight.
            CR = [(0, 252), (252, 36)]
            convP = []
            vS = []
            for mc in range(4):
                p = psum.tile([128, 288], F32, space="PSUM", tag="convP", bufs=4, name="convP")
                t = (pool.tile([128, 288], F32, tag=f"vS{mc-2}", name=f"vS{mc-2}")
                     if mc >= 2 else None)
                for r0, rn in CR:
                    for kc in range(2):
                        nc.tensor.matmul(
                            out=p[:, r0:r0 + rn],
                            lhsT=wsl("wcat", off=(kc * 4 + mc) * 128, width=128),
                            rhs=samT[kc][:, r0:r0 + rn],
                            start=(kc == 0), stop=(kc == 1))
                    if t is not None:
                        nc.scalar.copy(out=t[:, r0:r0 + rn], in_=p[:, r0:r0 + rn])
                convP.append(p)
                if t is not None:
                    vS.append(t)
            if debug:
                t = pool.tile([128, 288], F32)
                nc.scalar.copy(out=t[:], in_=convP[0][:])
                nc.sync.dma_start(out=dbg["d_conv0"][:], in_=t[:])

            pqS = mlp2(qse, 36, "wq1", "bq1", "wq2", "bq2", "q")
            for mc in range(2):
                nc.vector.tensor_tensor(out=pqS[mc][:], in0=pqS[mc][:],
                                        in1=qsT[mc], op=OP.mult)

            # ---- 12. sim = scaled per-head dots via selection matmuls
            simP = psum.tile([8, 288], F32, space="PSUM", tag="simP", bufs=1, name="simP")
            pairs = [(convP[0], cqS[0], "s0"), (convP[1], cqS[1], "s1"),
                     (pkS[0], pqS[0], "s0"), (pkS[1], pqS[1], "s1")]
            tmps = [pool.tile([128, 288], F32R, tag=f"tmp{i}", name=f"tmp{i}")
                    for i in range(4)]
            for r0, rn in CR:
                ng = rn // 36
                for i, (kpart, qpart, sname) in enumerate(pairs):
                    tmp = tmps[i]
                    qap = qpart[:]
                    ka = kpart[:]
                    ta = tmp[:]
                    nc.vector.tensor_tensor(
                        out=AP(ta.tensor, ta.offset + r0, [ta.ap[0], [36, ng], [1, 36]]),
                        in0=AP(ka.tensor, ka.offset + r0, [ka.ap[0], [36, ng], [1, 36]]),
                        in1=AP(qap.tensor, qap.offset, [qap.ap[0], [0, ng], [1, 36]]),
                        op=OP.mult)
                    nc.tensor.matmul(out=simP[:, r0:r0 + rn], lhsT=wsl(sname, width=8),
                                     rhs=tmp[:, r0:r0 + rn],
                                     start=(i == 0), stop=(i == 3))
            if debug:
                t = pool.tile([8, 288], F32)
                nc.vector.tensor_copy(out=t[:], in_=simP[:])
                nc.sync.dma_start(out=dbg["d_sim"][:], in_=t[:])

            # ---- 13+14. softmax (deferred normalization) + weighted values.
            # exp -> unnormalized attn; head-expand exp and the per-(h,q)
            # reciprocal separately, normalize the reduced (128, 36) output.
            # (|sim| <= ~3 so exp without max-subtract is safe; softmax is
            # shift-invariant so the result is identical.)
            ex = pool.tile([8, 288], F32R)
            nc.scalar.activation(out=ex[:], in_=simP[:], func=AF.Exp)
            sm = pool.tile([8, 36], F32)
            nc.vector.reduce_sum(out=sm[:], in_=view3(ex[:], [[1, 36], [36, 8]]),
                                 axis=mybir.AxisListType.X)
            rc = pool.tile([8, 36], F32R)
            with nc.allow_low_precision(reason="f32r keeps full fp32 range; "
                                        "mantissa rounding is ~1e-4 rel"):
                nc.vector.reciprocal(out=rc[:], in_=sm[:])
            if debug:
                at = pool.tile([8, 288], F32)
                rca = rc[:]
                nc.vector.tensor_tensor(
                    out=view3(at[:], [[1, 36], [36, 8]]),
                    in0=view3(ex[:], [[1, 36], [36, 8]]),
                    in1=AP(rca.tensor, rca.offset, [rca.ap[0], [1, 36], [0, 8]]),
                    op=OP.mult)
                nc.sync.dma_start(out=dbg["d_at"][:], in_=at[:])

            avT = []
            for fc in range(2):
                ae = psum.tile([128, 288], F32, space="PSUM", tag="psA", bufs=3, name="aeP")
                nc.tensor.matmul(out=ae[:], lhsT=wsl(f"e{fc}", rows=8, width=128),
                                 rhs=ex[:], start=True, stop=True)
                pr = pool.tile([128, 288], F32, tag=f"pr{fc}", name=f"pr{fc}")
                nc.vector.tensor_tensor(out=pr[:], in0=vS[fc][:], in1=ae[:],
                                        op=OP.mult)
                avu = pool.tile([128, 36], F32, tag=f"avu{fc}", name=f"avu{fc}")
                nc.vector.reduce_sum(out=avu[:], in_=view3(pr[:], [[1, 36], [36, 8]]),
                                     axis=mybir.AxisListType.X)
                re = psum.tile([128, 288], F32, space="PSUM", tag="psA", bufs=3, name="reP")
                nc.tensor.matmul(out=re[:, :36], lhsT=wsl(f"e{fc}", rows=8, width=128),
                                 rhs=rc[:], start=True, stop=True)
                av = pool.tile([128, 36], F32, tag=f"avT{fc}", name=f"avT{fc}")
                nc.vector.tensor_tensor(out=av[:], in0=avu[:], in1=re[:, :36],
                                        op=OP.mult)
                avT.append(av)
            if debug:
                nc.sync.dma_start(out=dbg["d_av0"][:], in_=avT[0][:])

            # ---- 15. out = attn_out @ W_out + b_out + identity (single DMA)
            oT = pool.tile([128, 72], F32)
            for mc in range(2):
                p = psum.tile([128, 288], F32, space="PSUM", tag="psA", bufs=3, name="oP")
                for kc in range(2):
                    nc.tensor.matmul(
                        out=p[:, :36], lhsT=wsl("wout", off=(kc * 2 + mc) * 128, width=128),
                        rhs=avT[kc][:], start=(kc == 0), stop=(kc == 1))
                nc.scalar.activation(out=oT[:, mc * 36:(mc + 1) * 36],
                                     in_=p[:, :36], func=AF.Identity,
                                     bias=wsl("bout", off=mc, width=1))
                nc.vector.tensor_tensor(out=oT[:, mc * 36:(mc + 1) * 36],
                                        in0=oT[:, mc * 36:(mc + 1) * 36],
                                        in1=deT[mc], op=OP.add)
            ota = oT[:]
            oda = out[:]
            nc.sync.dma_start(
                out=AP(oda.tensor, oda.offset, [[36, 128], [128 * 36, 2], [1, 36]]),
                in_=AP(ota.tensor, ota.offset, [[72, 128], [36, 2], [1, 36]]))

    return nc


# ------------------------------------------------------------------- driver

def make_in_maps(dec_embed, bev_feat, query_scale, ref_points, weights):
    wbs = pack_wblobs(weights)
    in_maps = []
    for c in range(8):
        b, kh = c // 2, c % 2
        hwc = bev_feat[b].transpose(1, 2, 0).reshape(H * W, 256)
        bev_hwc = np.zeros((H * W, 512), np.float32)
        bev_hwc[:, 0:256] = hwc
        bev_hwc[:(H - 1) * W, 256:512] = hwc[W:]
        bev_hwc = np.ascontiguousarray(bev_hwc)
        xb = pack_xblob(dec_embed, query_scale, ref_points, b, 3 * kh)
        in_maps.append({"bev": bev_hwc, "wblA0": wbs["A0"], "wblA": wbs["A"],
                        "wblR": wbs["R"], "wblB": wbs["B"], "xbl": xb})
    return in_maps


def assemble_output(results, dec_dtype=np.float32):
    out = np.zeros((K, B, T, DIM), np.float32)
    for c in range(8):
        b, kh = c // 2, c % 2
        oc = results[c]["out"]                     # (256, 36)
        out[3 * kh:3 * kh + 3, b] = oc.T.reshape(3, T, DIM)
    return out


_WNAMES = ["W_con_q", "b_con_q", "W_con_k", "W_v", "Wq1", "bq1", "Wq2", "bq2",
           "Wk1", "bk1", "Wk2", "bk2", "Wo1", "bo1", "Wo2", "bo2",
           "W_out", "b_out"]


def kernel(**inputs):
    from concourse.bass_utils import run_bass_kernel_spmd
    dec_embed = np.asarray(inputs["dec_embed"], np.float32)
    bev_feat = np.asarray(inputs["bev_feat"], np.float32)
    query_scale = np.asarray(inputs["query_scale"], np.float32)
    ref_points = np.asarray(inputs["ref_points"], np.float32)
    weights = {n: np.asarray(inputs[n], np.float32) for n in _WNAMES}

    nc = build_nc(sim_mode=False, debug=False)
    split_multiwaits(nc)
    in_maps = make_in_maps(dec_embed, bev_feat, query_scale, ref_points, weights)
    res = run_bass_kernel_spmd(nc, in_maps, list(range(8)))
    return assemble_output(res.results)



# revision 8
# speedup vs baseline: 1.1876x; 1.1876x over previous
"""BEV deformable cross-attention kernel for 8 Trainium2 NeuronCores.

Strategy (per core): data-parallel over (B x K-half): core c handles batch
b = c//2 and modes k in {3*(c%2) .. +3}, i.e. 36 queries, 288 sample points.

Key algebraic move: grid_sample(conv1x1(bev)) == conv1x1(grid_sample(bev)),
so instead of materializing the two full (256,200,200) conv maps we gather
only the 4 bilinear corners of the 288 sample points from a host-transposed
HWC bf16 copy of bev_feat (channels contiguous per pixel -> 1KB indirect
reads), interpolate in 256-d, then apply the 1x1 convs to 288 vectors.

v2 speedups over the first working kernel:
  - all weights + matmul activations in bf16 (1 cyc/row PE, half LDWEIGHTS)
  - qse sine phases merged into the kse phase matmul (f32r, 324 cols)
  - gathers read bf16 (half the scattered HBM bytes)
  - bilinear via fused scalar_tensor_tensor, split over ACT+DVE
  - input DMAs reordered + spread over the two HWDGE queues (sync, scalar)
  - PE p-state pre-ramp with dummy matmuls during the weight-DMA wait
"""
import numpy as np
import ml_dtypes

import concourse.bass as bass
import concourse.mybir as mybir
import concourse.tile as tile_mod
from concourse.bass import AP, IndirectOffsetOnAxis

F32 = mybir.dt.float32
F32R = mybir.dt.float32r
BF16 = mybir.dt.bfloat16
I32 = mybir.dt.int32
AF = mybir.ActivationFunctionType
OP = mybir.AluOpType
BFNP = ml_dtypes.bfloat16

# problem constants (hardcoded per contract)
K, B, T, DIM = 6, 4, 12, 256
H, W = 200, 200
HALF = 256
G = 8                      # offset groups == sample points per query
NQ = 3 * T                 # queries per core = 36
NPT = NQ * G               # points per core = 288
NPH = NPT + NQ             # phase-matmul cols = kse 288 + qse 36
OFFSET_SCALE = 4.0
PIX_SCALE = float(W / 102.4)          # 1.953125
PIX_BIAS = float(W / 2.0 - 0.5)       # 99.5
SCALE = 64 ** -0.5                    # 0.125
TWO_PI = float(2 * np.pi)
RC = float(3 * 2 ** 22)               # 1.5*2^23 rint magic constant
CHUNKS = [(0, 128), (128, 128), (256, 32)]   # point chunks (start, size)
N_RAMP = 10                # PE p-state pre-ramp dummy matmuls

# ---------------------------------------------------------------- blob layout


class Alloc:
    def __init__(self):
        self.pos = 0
        self.slices = {}

    def add(self, name, width):
        self.slices[name] = (self.pos, width)
        self.pos += width

    def __getitem__(self, name):
        return self.slices[name]


# bf16 weight blobs, split by first-use time
WBLOBS = {
    "W0": [("wconq", 512), ("de16", 72), ("qs16", 72), ("bdh", 512),
           ("wo2top", 2), ("wo2bot", 2), ("ident16", 128)],
    "W1": [("wk1", 512), ("wk2", 512), ("wcat", 1024)],
    "W2": [("wq1", 512), ("wq2", 512), ("wout", 512),
           ("s0", 8), ("s1", 8), ("e0", 128), ("e1", 128)],
}
# fp32 per-core blob: geometry + biases
XF_ITEMS = [("deT", 72), ("bpm", 6), ("bconq", 2), ("bo1rep", 1), ("bo2", 1),
            ("bq1", 2), ("bq2", 2), ("bk1", 2), ("bk2", 2), ("bout", 2),
            ("sc4pm", 2)]
# f32r per-core blob: phase freqs + phase rhs rows + tiny f32r identity
XR_ITEMS = [("fk5y", 128), ("fk5x", 128), ("rpk", NPH), ("idr", 2)]


def blob_layout(items):
    a = Alloc()
    for nm, wd in items:
        a.add(nm, wd)
    return a


WLAYS = {w: blob_layout(items) for w, items in WBLOBS.items()}
XFLAY = blob_layout(XF_ITEMS)
XRLAY = blob_layout(XR_ITEMS)
NAME2BLOB = {nm: which for which, items in WBLOBS.items() for nm, _ in items}


def _put_mm(dst, lay, name, w256):
    """(256, Mout) -> column blocks (kc, mc) of (128, 128)."""
    s, _ = lay[name]
    mcs = w256.shape[1] // 128
    for kc in range(2):
        for mc in range(mcs):
            blk = w256[kc * 128:(kc + 1) * 128, mc * 128:(mc + 1) * 128]
            off = (kc * mcs + mc) * 128
            dst[:, s + off: s + off + 128] = blk


def pack_wblobs(weights):
    """weights: dict of numpy arrays (original reference layouts).
    Returns shared (core-independent) bf16 blobs W1, W2 and a template for
    W0 (per-core de16/qs16 filled later)."""
    wbs = {w: np.zeros((128, WLAYS[w].pos), np.float32) for w in WBLOBS}

    def put(which, name, arr, rows=128):
        s, _ = WLAYS[which][name]
        wbs[which][:rows, s: s + arr.shape[1]] = arr

    _put_mm(wbs["W0"], WLAYS["W0"], "wconq", weights["W_con_q"])
    # block-diag Wo1: j = cc*2+h2 covers groups (2j, 2j+1)
    s, _ = WLAYS["W0"]["bdh"]
    wo1 = weights["Wo1"]  # (32, 64)
    for j in range(4):
        blk = np.zeros((128, 128), np.float32)
        if j % 2 == 0:
            blk[0:32, 0:64] = wo1
            blk[32:64, 64:128] = wo1
        else:
            blk[64:96, 0:64] = wo1
            blk[96:128, 64:128] = wo1
        wbs["W0"][:, s + j * 128: s + (j + 1) * 128] = blk
    wo2 = weights["Wo2"]  # (64, 2)
    top = np.zeros((128, 2), np.float32); top[0:64] = wo2
    bot = np.zeros((128, 2), np.float32); bot[64:128] = wo2
    put("W0", "wo2top", top); put("W0", "wo2bot", bot)
    put("W0", "ident16", np.eye(128, dtype=np.float32))

    _put_mm(wbs["W1"], WLAYS["W1"], "wk1", weights["Wk1"])
    _put_mm(wbs["W1"], WLAYS["W1"], "wk2", weights["Wk2"])
    wcat = np.concatenate([weights["W_con_k"], weights["W_v"]], axis=1)
    _put_mm(wbs["W1"], WLAYS["W1"], "wcat", wcat)

    _put_mm(wbs["W2"], WLAYS["W2"], "wq1", weights["Wq1"])
    _put_mm(wbs["W2"], WLAYS["W2"], "wq2", weights["Wq2"])
    _put_mm(wbs["W2"], WLAYS["W2"], "wout", weights["W_out"])
    d = np.arange(128)
    s0 = np.zeros((128, 8), np.float32); s0[d, d // 32] = SCALE
    s1 = np.zeros((128, 8), np.float32); s1[d, 4 + d // 32] = SCALE
    put("W2", "s0", s0); put("W2", "s1", s1)
    e0 = np.zeros((8, 128), np.float32); e0[d // 32, d] = 1.0
    e1 = np.zeros((8, 128), np.float32); e1[4 + d // 32, d] = 1.0
    put("W2", "e0", e0, rows=8); put("W2", "e1", e1, rows=8)
    return wbs


def pack_xblobs(weights, dec_embed, query_scale, ref_points, b, k0, w0_tmpl):
    """Per-core blobs: W0 (bf16, with de16/qs16), XF (fp32), XR (f32r=f32)."""
    de = dec_embed[k0:k0 + 3, b].reshape(NQ, DIM)       # (36, 256)
    qs = query_scale[k0:k0 + 3, b].reshape(NQ, DIM)
    rp = ref_points[k0:k0 + 3, b].reshape(NQ, 2)

    w0 = w0_tmpl.copy()
    s, _ = WLAYS["W0"]["de16"]
    w0[:, s: s + 36] = de.T[:128]
    w0[:, s + 36: s + 72] = de.T[128:]
    s, _ = WLAYS["W0"]["qs16"]
    w0[:, s: s + 36] = qs.T[:128]
    w0[:, s + 36: s + 72] = qs.T[128:]

    xf = np.zeros((128, XFLAY.pos), np.float32)

    def putf(name, arr, rows=128):
        s, _ = XFLAY[name]
        xf[:rows, s: s + arr.shape[1]] = arr

    putf("deT", np.concatenate([de.T[:128], de.T[128:]], axis=1))
    rpe = np.tile(rp.T, (1, 8))                         # g-major: col = g*36+q
    bx = PIX_SCALE * rpe[0] + PIX_BIAS
    by = -PIX_SCALE * rpe[1] + PIX_BIAS
    s, _ = XFLAY["bpm"]
    for c, (c0, cn) in enumerate(CHUNKS):
        xf[:cn, s + 2 * c] = bx[c0:c0 + cn]
        xf[:cn, s + 2 * c + 1] = by[c0:c0 + cn]
    putf("bconq", weights["b_con_q"].reshape(2, 128).T)
    putf("bo1rep", np.tile(weights["bo1"], 2)[:, None])
    putf("bo2", weights["bo2"][:, None], rows=2)
    putf("bq1", weights["bq1"].reshape(2, 128).T)
    putf("bq2", weights["bq2"].reshape(2, 128).T)
    putf("bk1", weights["bk1"].reshape(2, 128).T)
    putf("bk2", weights["bk2"].reshape(2, 128).T)
    putf("bout", weights["b_out"].reshape(2, 128).T)
    putf("sc4pm", np.tile(np.array([[4 * PIX_SCALE, -4 * PIX_SCALE]],
                                   np.float32), (128, 1)))

    xr = np.zeros((128, XRLAY.pos), np.float32)
    i64 = np.arange(128) // 2
    freq = (TWO_PI / (10000.0 ** (i64 / 64.0))).astype(np.float32)
    shift = np.where(np.arange(128) % 2 == 1, np.pi / 2, 0.0).astype(np.float32)
    fk5x = np.zeros((5, 128), np.float32)
    fk5x[0] = 4 * freq; fk5x[2] = freq; fk5x[4] = shift
    fk5y = np.zeros((5, 128), np.float32)
    fk5y[1] = 4 * freq; fk5y[3] = freq; fk5y[4] = shift
    s, _ = XRLAY["fk5x"]; xr[:5, s: s + 128] = fk5x
    s, _ = XRLAY["fk5y"]; xr[:5, s: s + 128] = fk5y
    # rpk rows 0:5 -> kseRhs rows 0:5: rows 0:2 zero (tanh overwrites the kse
    # cols on device), rows 2:5 = [rpexp_x; rpexp_y; ones] for kse cols and
    # [rp_x; rp_y; ones] for the 36 qse cols
    s, _ = XRLAY["rpk"]
    xr[2, s: s + 288] = rpe[0]
    xr[3, s: s + 288] = rpe[1]
    xr[2, s + 288: s + 324] = rp[:, 0]
    xr[3, s + 288: s + 324] = rp[:, 1]
    xr[4, s: s + 324] = 1.0
    s, _ = XRLAY["idr"]
    xr[0, s] = 1.0; xr[1, s + 1] = 1.0
    return w0.astype(BFNP), xf, xr


# --------------------------------------------------------------- tile patches

def _split_drain_and_barrier(self, tick_clock, wait_clock):
    nc = self.nc
    drain_inst = nc.sync.drain()
    wait_clock.add_sem_waits(
        drain_inst.ins, tile_mod.ScopedClock({None: tick_clock.global_clock})
    )
    si = drain_inst.ins.sync_info
    waits = list(si.on_wait)
    if len(waits) > 1:
        si.on_wait = waits[:1]
        for i in range(1, len(waits)):
            extra = nc.sync.drain()
            extra.ins.sync_info = type(si)(on_wait=waits[i: i + 1], on_update=[])
    nc.all_engine_barrier()
    assert self.sems is not None
    popped = nc._tile_sem_poison_stack.pop()
    assert popped is self._sem_poison
    nc.clear_and_free_semaphores(list(self.sems.allocated().values()))


def split_multiwaits(nc):
    """walrus codegen supports a single sync-wait per instruction; split."""
    f = nc.m.functions[0]
    for blk in f.blocks:
        todo = [i for i in blk.instructions
                if i.sync_info is not None and len(i.sync_info.on_wait) > 1]
        for inst in todo:
            si = inst.sync_info
            waits = list(si.on_wait)
            nops = []
            for w in waits[:-1]:
                bi = nc.engines[inst.engine].nop(nofuse=True)
                ni = bi.ins
                for b2 in f.blocks:
                    if b2.instructions and b2.instructions[-1] is ni:
                        b2.instructions.pop()
                        break
                ni.sync_info = type(si)(on_wait=[w], on_update=[])
                nops.append(ni)
            si.on_wait = [waits[-1]]
            pos = blk.instructions.index(inst)
            blk.instructions[pos:pos] = nops


_PATCHED = False


def patch_tile():
    global _PATCHED
    if not _PATCHED:
        tile_mod.TileContext._drain_and_barrier = _split_drain_and_barrier
        _PATCHED = True


# ---------------------------------------------------------------- the kernel

def view3(ap, dims):
    """Build an AP view on top of a 2D tile AP: dims = [[step,count],...]
    applied after the partition dim (ap.ap[0] kept)."""
    return AP(ap.tensor, ap.offset, [ap.ap[0]] + dims)


def build_nc(sim_mode=False, debug=False):
    patch_tile()
    nc = bass.Bass("TRN2")

    # row-pair interleaved bf16: bev[y*W+x] = [feat(y,x) (256) | feat(y+1,x)]
    bev = nc.dram_tensor("bev", [H * W, 512], BF16, kind="ExternalInput")
    wbl = {w: nc.dram_tensor(f"wbl{w}", [128, WLAYS[w].pos], BF16,
                             kind="ExternalInput") for w in WBLOBS}
    xfl = nc.dram_tensor("xfl", [128, XFLAY.pos], F32, kind="ExternalInput")
    xrl = nc.dram_tensor("xrl", [128, XRLAY.pos], F32R, kind="ExternalInput")
    out = nc.dram_tensor("out", [256, NQ], F32, kind="ExternalOutput")

    dbg = {}
    if debug:
        for nm, shp, dt in [
            ("d_pix", [128, 2], F32),
            ("d_idx", [128, 1], I32), ("d_sam0", [128, 256], F32),
            ("d_sim", [8, 288], F32), ("d_at", [8, 288], F32),
            ("d_kse0", [128, NPH], F32), ("d_posk0", [128, 288], F32),
            ("d_conv0", [128, 288], F32), ("d_cq0", [128, 36], F32),
            ("d_h", [128, 144], F32), ("d_w40", [128, 4], F32),
            ("d_av0", [128, 36], F32), ("d_pq0", [128, 36], F32),
        ]:
            dbg[nm] = nc.dram_tensor(nm, shp, dt, kind="ExternalOutput")

    with tile_mod.TileContext(nc) as tc:
        with (
            tc.tile_pool(name="sbuf", bufs=1) as pool,
            tc.tile_pool(name="psum", bufs=1, space="PSUM") as psum,
        ):
            # ---- PE p-state pre-ramp: dummy matmuls during the weight DMA.
            zt = pool.tile([128, 128], BF16)
            nc.vector.memset(zt[:], 0.0)
            for r in range(N_RAMP):
                rp_ = psum.tile([128, 128], F32, space="PSUM", tag="psA",
                                bufs=3, name="rampP")
                nc.tensor.matmul(out=rp_[:], lhsT=zt[:], rhs=zt[:],
                                 start=True, stop=True)

            # ---- input DMAs: sync queue gets the early blobs, scalar the late
            wb = {w: pool.tile([128, WLAYS[w].pos], BF16, name=f"wb{w}")
                  for w in WBLOBS}
            xf = pool.tile([128, XFLAY.pos], F32)
            xr = pool.tile([128, XRLAY.pos], F32R)
            nc.sync.dma_start(out=wb["W0"][:], in_=wbl["W0"][:])
            nc.sync.dma_start(out=xf[:], in_=xfl[:])
            nc.sync.dma_start(out=xr[:], in_=xrl[:])
            nc.scalar.dma_start(out=wb["W1"][:], in_=wbl["W1"][:])
            nc.scalar.dma_start(out=wb["W2"][:], in_=wbl["W2"][:])

            # warm the {erf,tanh,gelu} activation table during the weight DMA
            wt = pool.tile([1, 1], F32)
            nc.vector.memset(wt[:], 0.0)
            warm = pool.tile([1, 1], F32)
            nc.scalar.activation(out=warm[:], in_=wt[:],
                                 func=AF.Sigmoid if sim_mode else AF.Gelu,
                                 bias=0.0)

            def wsl(name, rows=128, off=0, width=None):
                which = NAME2BLOB[name]
                s, wd = WLAYS[which][name]
                if width is None:
                    width = wd - off
                return wb[which][0:rows, s + off: s + off + width]

            def xfsl(name, rows=128, off=0, width=None):
                s, wd = XFLAY[name]
                if width is None:
                    width = wd - off
                return xf[0:rows, s + off: s + off + width]

            def xrsl(name, rows=128, off=0, width=None):
                s, wd = XRLAY[name]
                if width is None:
                    width = wd - off
                return xr[0:rows, s + off: s + off + width]

            de16 = [wsl("de16", off=mc * 36, width=36) for mc in range(2)]
            qs16 = [wsl("qs16", off=mc * 36, width=36) for mc in range(2)]
            deT = [xfsl("deT", off=mc * 36, width=36) for mc in range(2)]

            # kse+qse phase rhs: rows 0:2 tanh (device), rows 2:5 host
            kseRhs = pool.tile([5, NPH], F32R)
            s_rpk, _ = XRLAY["rpk"]
            nc.sync.dma_start(out=kseRhs[0:5, :], in_=xrl[0:5, s_rpk:s_rpk + NPH])

            # ---- 1. con_q = de @ W_con_q + b  (feature-major, 2 chunks)
            cqS = []
            for mc in range(2):
                p = psum.tile([128, 288], F32, space="PSUM", tag="psA", bufs=3, name="cqP")
                for kc in range(2):
                    nc.tensor.matmul(
                        out=p[:, :36], lhsT=wsl("wconq", off=(kc * 2 + mc) * 128, width=128),
                        rhs=de16[kc], start=(kc == 0), stop=(kc == 1))
                t = pool.tile([128, 36], BF16, tag=f"cqS{mc}")
                nc.scalar.activation(out=t[:], in_=p[:, :36], func=AF.Identity,
                                     bias=xfsl("bconq", off=mc, width=1))
                cqS.append(t)
            if debug:
                t = pool.tile([128, 36], F32)
                nc.vector.tensor_copy(out=t[:], in_=cqS[0][:])
                nc.sync.dma_start(out=dbg["d_cq0"][:], in_=t[:])

            # ---- 2. h = gelu(grouped con_q @ Wo1 + bo1): 4 block-diag mms
            hP = psum.tile([128, 288], F32, space="PSUM", tag="psA", bufs=3, name="hP")
            for j in range(4):
                cc = j // 2
                nc.tensor.matmul(
                    out=hP[:, j * 36:(j + 1) * 36],
                    lhsT=wsl("bdh", off=j * 128, width=128),
                    rhs=cqS[cc][:], start=True, stop=True)
            hS = pool.tile([128, 144], BF16)
            if sim_mode:
                hx = pool.tile([128, 144], F32)
                nc.scalar.activation(out=hx[:], in_=hP[:, :144], func=AF.Identity,
                                     bias=xfsl("bo1rep"))
                he = pool.tile([128, 144], F32)
                nc.scalar.activation(out=he[:], in_=hx[:], func=AF.Sigmoid,
                                     scale=float(1 / np.sqrt(2)), bias=0.0)
                nc.vector.tensor_scalar(out=he[:], in0=he[:], scalar1=0.5,
                                        scalar2=0.5, op0=OP.mult, op1=OP.add)
                nc.vector.tensor_tensor(out=hS[:], in0=hx[:], in1=he[:], op=OP.mult)
            else:
                nc.scalar.activation(out=hS[:], in_=hP[:, :144], func=AF.Gelu,
                                     bias=xfsl("bo1rep"))
            if debug:
                t = pool.tile([128, 144], F32)
                nc.vector.tensor_copy(out=t[:], in_=hS[:])
                nc.sync.dma_start(out=dbg["d_h"][:], in_=t[:])

            # ---- 3. offsets: 2 matmuls (even/odd groups) into strided psum;
            # tanh lands in rows 0:2 of the f32r kse-rhs tile.
            offP = psum.tile([2, 288], F32, space="PSUM", tag="psA", bufs=3, name="offP")
            for m, wn in [(0, "wo2top"), (1, "wo2bot")]:
                nc.tensor.matmul(
                    out=offP[:, m * 144:(m + 1) * 144],
                    lhsT=wsl(wn, width=2),
                    rhs=hS[:], start=True, stop=True)
            kra = kseRhs[0:2, 0:288]
            opa2 = offP[:]
            nc.scalar.activation(
                out=AP(kra.tensor, kra.offset,
                       [kra.ap[0], [72, 4], [36, 2], [1, 36]]),
                in_=AP(opa2.tensor, opa2.offset,
                       [opa2.ap[0], [36, 4], [144, 2], [1, 36]]),
                func=AF.Tanh, bias=xfsl("bo2", rows=2, width=1))
            # warm the trig table right after tanh (gelu table no longer needed)
            wts = pool.tile([1, 1], F32)
            nc.scalar.activation(out=wts[:], in_=wt[:], func=AF.Sin)

            # ---- 4+5. transpose tanh to point-major, then per-point
            # geometry; gathers issue per chunk as soon as idx is ready.
            idxI, frs, gA = [], [], []
            s_bpm, _ = XFLAY["bpm"]
            for c, (c0, cn) in enumerate(CHUNKS):
                tp = psum.tile([128, 2], F32R, space="PSUM", tag="psA", bufs=3, name="tpP")
                nc.tensor.transpose(out=tp[:cn, :], in_=kseRhs[0:2, c0:c0 + cn],
                                    identity=xrsl("idr", rows=2, width=2))
                pix = pool.tile([128, 2], F32, tag=f"pix{c}", name=f"pix{c}")
                nc.vector.tensor_tensor(out=pix[:cn, :], in0=tp[:cn, :],
                                        in1=xfsl("sc4pm", rows=cn, width=2),
                                        op=OP.mult)
                nc.vector.tensor_tensor(
                    out=pix[:cn, :], in0=pix[:cn, :],
                    in1=xf[0:cn, s_bpm + 2 * c: s_bpm + 2 * c + 2], op=OP.add)
                f0 = pool.tile([128, 2], F32, tag=f"f0{c}", name=f"f0{c}")
                nc.vector.tensor_scalar(out=f0[:cn, :], in0=pix[:cn, :],
                                        scalar1=-0.5, scalar2=float(RC),
                                        op0=OP.add, op1=OP.add)
                nc.vector.tensor_scalar(out=f0[:cn, :], in0=f0[:cn, :],
                                        scalar1=float(-RC), scalar2=None,
                                        op0=OP.add)
                fr = pool.tile([128, 2], F32, tag=f"fr{c}", name=f"fr{c}")
                nc.vector.tensor_tensor(out=fr[:cn, :], in0=pix[:cn, :],
                                        in1=f0[:cn, :], op=OP.subtract)
                frs.append(fr)
                idf = pool.tile([128, 1], F32, tag=f"idf{c}", name=f"idf{c}")
                nc.vector.tensor_scalar(out=idf[:cn, :], in0=f0[:cn, 1:2],
                                        scalar1=float(W), scalar2=None,
                                        op0=OP.mult)
                nc.vector.tensor_tensor(out=idf[:cn, :], in0=idf[:cn, :],
                                        in1=f0[:cn, 0:1], op=OP.add)
                ii = pool.tile([128, 1], I32, tag=f"idxI{c}", name=f"idxI{c}")
                nc.vector.tensor_copy(out=ii[:cn, :], in_=idf[:cn, :])
                idxI.append(ii)
                ga = pool.tile([128, 1024], BF16, tag=f"gA{c}", name=f"gA{c}")
                nc.gpsimd.indirect_dma_start(
                    out=ga[:cn, :], out_offset=None, in_=bev[:],
                    in_offset=IndirectOffsetOnAxis(ap=ii[:cn, :], axis=0))
                gA.append(ga)
                if debug and c == 0:
                    nc.sync.dma_start(out=dbg["d_pix"][:], in_=pix[:])
                    nc.sync.dma_start(out=dbg["d_idx"][:], in_=ii[:])

            # ---- 6. kse (+merged qse) sine embedding, f32r phases.
            # rows 0:64 (high freqs) need range reduction; rows 64:128 pass.
            st = []
            for half, lhs in enumerate(["fk5y", "fk5x"]):
                ph = psum.tile([128, NPH], F32, space="PSUM", tag="psA", bufs=3, name="phP")
                nc.tensor.matmul(out=ph[:], lhsT=xrsl(lhs, rows=5),
                                 rhs=kseRhs[:], start=True, stop=True)
                NR = 96   # rows needing range reduction (high freqs)
                m1t = pool.tile([NR, NPH], F32, tag=f"sm1{half}")
                nc.scalar.activation(out=m1t[:], in_=ph[0:NR, :], func=AF.Copy,
                                     scale=float(1.0 / TWO_PI), bias=float(RC))
                k2t = pool.tile([NR, NPH], F32, tag=f"sk2{half}")
                nc.vector.tensor_scalar(out=k2t[:], in0=m1t[:],
                                        scalar1=float(-RC), scalar2=float(-TWO_PI),
                                        op0=OP.add, op1=OP.mult)
                yt = pool.tile([128, NPH], F32, tag=f"sy{half}")
                nc.vector.tensor_tensor(out=yt[0:NR, :], in0=ph[0:NR, :],
                                        in1=k2t[:], op=OP.add)
                nc.vector.tensor_scalar(out=yt[0:NR, :], in0=yt[0:NR, :],
                                        scalar1=float(np.pi),
                                        scalar2=float(-np.pi),
                                        op0=OP.min, op1=OP.max)
                nc.vector.tensor_copy(out=yt[96:128, :], in_=ph[96:128, :])
                s_ = pool.tile([128, NPH], BF16, tag=f"se{half}")
                nc.scalar.activation(out=s_[:], in_=yt[:], func=AF.Sin)
                st.append(s_)
            if debug:
                t = pool.tile([128, NPH], F32)
                nc.vector.tensor_copy(out=t[:], in_=st[0][:])
                nc.sync.dma_start(out=dbg["d_kse0"][:], in_=t[:])

            # ---- helpers: 2-layer MLP relu(x@W1+b1)@W2+b2, feature-major
            def mlp2(inp2, n, wn1, bn1, wn2, bn2, tag):
                mid = []
                for mc in range(2):
                    p = psum.tile([128, 288], F32, space="PSUM", tag="psA", bufs=3, name="m1P")
                    for kc in range(2):
                        nc.tensor.matmul(
                            out=p[:, :n], lhsT=wsl(wn1, off=(kc * 2 + mc) * 128, width=128),
                            rhs=inp2[kc], start=(kc == 0), stop=(kc == 1))
                    t = pool.tile([128, n], BF16, tag=f"m1S{tag}{mc}")
                    nc.scalar.activation(out=t[:], in_=p[:, :n], func=AF.Relu,
                                         bias=xfsl(bn1, off=mc, width=1))
                    mid.append(t)
                outs = []
                for mc in range(2):
                    p = psum.tile([128, 288], F32, space="PSUM", tag="psA", bufs=3, name="m2P")
                    for kc in range(2):
                        nc.tensor.matmul(
                            out=p[:, :n], lhsT=wsl(wn2, off=(kc * 2 + mc) * 128, width=128),
                            rhs=mid[kc][:], start=(kc == 0), stop=(kc == 1))
                    t = pool.tile([128, n], BF16, tag=f"m2S{tag}{mc}")
                    nc.scalar.activation(out=t[:], in_=p[:, :n], func=AF.Identity,
                                         bias=xfsl(bn2, off=mc, width=1))
                    outs.append(t)
                return outs

            # ---- 7. pos_k MLP on the kse cols
            kse_in = [st[0][:, 0:288], st[1][:, 0:288]]
            pkS = mlp2(kse_in, 288, "wk1", "bk1", "wk2", "bk2", "k")
            if debug:
                t = pool.tile([128, 288], F32)
                nc.vector.tensor_copy(out=t[:], in_=pkS[0][:])
                nc.sync.dma_start(out=dbg["d_posk0"][:], in_=t[:])
            # prefetch the exp table once the Sin burst is done
            wt2 = pool.tile([1, 1], F32)
            nc.scalar.activation(out=wt2[:], in_=wt[:], func=AF.Exp)

            # ---- 8. bilinear weights + combine -> sampled (point-major bf16)
            # gather quarters: [c00 | c01(y+1) | c10(x+1) | c11]; w4 cols
            # [w00, w10, w01, w11] -> quarter j uses w4 col [0, 2, 1, 3][j]
            w4 = []
            for c, (c0, cn) in enumerate(CHUNKS):
                fr = frs[c]
                wxp = pool.tile([128, 2], F32, tag=f"wxp{c}", name=f"wxp{c}")
                nc.vector.tensor_scalar(out=wxp[:cn, 0:1], in0=fr[:cn, 0:1],
                                        scalar1=-1.0, scalar2=1.0,
                                        op0=OP.mult, op1=OP.add)
                nc.vector.tensor_copy(out=wxp[:cn, 1:2], in_=fr[:cn, 0:1])
                wyp = pool.tile([128, 2], F32, tag=f"wyp{c}", name=f"wyp{c}")
                nc.vector.tensor_scalar(out=wyp[:cn, 0:1], in0=fr[:cn, 1:2],
                                        scalar1=-1.0, scalar2=1.0,
                                        op0=OP.mult, op1=OP.add)
                nc.vector.tensor_copy(out=wyp[:cn, 1:2], in_=fr[:cn, 1:2])
                w4c = pool.tile([128, 4], F32, tag=f"w4{c}", name=f"w4{c}")
                wxa = wxp[:cn, :]
                wya = wyp[:cn, :]
                nc.vector.tensor_tensor(
                    out=view3(w4c[:cn, :], [[2, 2], [1, 2]]),
                    in0=AP(wxa.tensor, wxa.offset, [wxa.ap[0], [0, 2], [1, 2]]),
                    in1=AP(wya.tensor, wya.offset, [wya.ap[0], [1, 2], [0, 2]]),
                    op=OP.mult)
                w4.append(w4c)
            if debug:
                nc.sync.dma_start(out=dbg["d_w40"][:], in_=w4[0][:])

            sam = []
            for c, (c0, cn) in enumerate(CHUNKS):
                g = gA[c]
                a = pool.tile([128, 256], BF16, tag=f"ba{c}", name=f"ba{c}")
                b_ = pool.tile([128, 256], BF16, tag=f"bb{c}", name=f"bb{c}")
                sm = pool.tile([128, 256], BF16, tag=f"sam{c}", name=f"sam{c}")
                # a = c00*w00 ; b = c10*w01 (ACT, per-partition scale)
                nc.scalar.activation(out=a[:cn, :], in_=g[:cn, 0:256],
                                     func=AF.Copy, scale=w4[c][:cn, 0:1])
                nc.scalar.activation(out=b_[:cn, :], in_=g[:cn, 256:512],
                                     func=AF.Copy, scale=w4[c][:cn, 2:3])
                # a += c01*w10 ; b += c11*w11 (fused DVE)
                nc.vector.scalar_tensor_tensor(
                    out=a[:cn, :], in0=g[:cn, 512:768], scalar=w4[c][:cn, 1:2],
                    in1=a[:cn, :], op0=OP.mult, op1=OP.add)
                nc.vector.scalar_tensor_tensor(
                    out=b_[:cn, :], in0=g[:cn, 768:1024], scalar=w4[c][:cn, 3:4],
                    in1=b_[:cn, :], op0=OP.mult, op1=OP.add)
                nc.vector.tensor_tensor(out=sm[:cn, :], in0=a[:cn, :],
                                        in1=b_[:cn, :], op=OP.add)
                sam.append(sm)
            if debug:
                t = pool.tile([128, 256], F32)
                nc.vector.tensor_copy(out=t[:], in_=sam[0][:])
                nc.sync.dma_start(out=dbg["d_sam0"][:], in_=t[:])

            # ---- 9. transpose sampled to feature-major (256, 288) = 2 tiles
            samT = [pool.tile([128, 288], BF16, tag=f"samT{fc}", name=f"samT{fc}")
                    for fc in range(2)]
            for c, (c0, cn) in enumerate(CHUNKS):
                for fc in range(2):
                    tp = psum.tile([128, 128], BF16, space="PSUM", tag="psA", bufs=3, name="samtp")
                    nc.tensor.transpose(out=tp[:, :cn],
                                        in_=sam[c][:cn, fc * 128:(fc + 1) * 128],
                                        identity=wsl("ident16", rows=cn, width=cn))
                    nc.vector.tensor_copy(out=samT[fc][:, c0:c0 + cn], in_=tp[:, :cn])

            # ---- 10. conv: con_k / v = sampled @ [W_con_k | W_v]
            convP = []
            vS = []
            for mc in range(4):
                p = psum.tile([128, 288], F32, space="PSUM", tag="convP", bufs=4, name="convP")
                for kc in range(2):
                    nc.tensor.matmul(
                        out=p[:],
                        lhsT=wsl("wcat", off=(kc * 4 + mc) * 128, width=128),
                        rhs=samT[kc][:], start=(kc == 0), stop=(kc == 1))
                convP.append(p)
                if mc >= 2:
                    t = pool.tile([128, 288], BF16, tag=f"vS{mc-2}", name=f"vS{mc-2}")
                    nc.vector.tensor_copy(out=t[:], in_=p[:])
                    vS.append(t)
            if debug:
                t = pool.tile([128, 288], F32)
                nc.vector.tensor_copy(out=t[:], in_=convP[0][:])
                nc.sync.dma_start(out=dbg["d_conv0"][:], in_=t[:])

            # ---- 11. pos_q MLP on the merged qse cols, then * query_scale
            qse_in = [st[0][:, 288:NPH], st[1][:, 288:NPH]]
            pqS = mlp2(qse_in, 36, "wq1", "bq1", "wq2", "bq2", "q")
            for mc in range(2):
                nc.vector.tensor_tensor(out=pqS[mc][:], in0=pqS[mc][:],
                                        in1=qs16[mc], op=OP.mult)
            if debug:
                t = pool.tile([128, 36], F32)
                nc.vector.tensor_copy(out=t[:], in_=pqS[0][:])
                nc.sync.dma_start(out=dbg["d_pq0"][:], in_=t[:])

            # ---- 12. sim = scaled per-head dots via selection matmuls
            simP = psum.tile([8, 288], F32, space="PSUM", tag="simP", bufs=1, name="simP")
            pairs = [(convP[0], cqS[0], "s0"), (convP[1], cqS[1], "s1"),
                     (pkS[0], pqS[0], "s0"), (pkS[1], pqS[1], "s1")]
            for i, (kpart, qpart, sname) in enumerate(pairs):
                tmp = pool.tile([128, 288], BF16, tag=f"tmp{i}", name=f"tmp{i}")
                qap = qpart[:]
                ka = kpart[:]
                ta = tmp[:]
                nc.vector.tensor_tensor(
                    out=AP(ta.tensor, ta.offset, [ta.ap[0], [36, 8], [1, 36]]),
                    in0=AP(ka.tensor, ka.offset, [ka.ap[0], [36, 8], [1, 36]]),
                    in1=AP(qap.tensor, qap.offset, [qap.ap[0], [0, 8], [1, 36]]),
                    op=OP.mult)
                nc.tensor.matmul(out=simP[:], lhsT=wsl(sname, width=8),
                                 rhs=tmp[:],
                                 start=(i == 0), stop=(i == 3))
            if debug:
                t = pool.tile([8, 288], F32)
                nc.vector.tensor_copy(out=t[:], in_=simP[:])
                nc.sync.dma_start(out=dbg["d_sim"][:], in_=t[:])

            # ---- 13+14. softmax (deferred normalization) + weighted values.
            # (|sim| <= ~3 so exp without max-subtract is safe; softmax is
            # shift-invariant so the result is identical.)
            ex = pool.tile([8, 288], BF16)
            nc.scalar.activation(out=ex[:], in_=simP[:], func=AF.Exp)
            sm = pool.tile([8, 36], F32)
            nc.vector.reduce_sum(out=sm[:], in_=view3(ex[:], [[1, 36], [36, 8]]),
                                 axis=mybir.AxisListType.X)
            rc = pool.tile([8, 36], BF16)
            with nc.allow_low_precision(reason="bf16 reciprocal: ~4e-3 rel on "
                                        "the softmax norm, within tolerance"):
                nc.vector.reciprocal(out=rc[:], in_=sm[:])
            if debug:
                at = pool.tile([8, 288], F32)
                rca = rc[:]
                nc.vector.tensor_tensor(
                    out=view3(at[:], [[1, 36], [36, 8]]),
                    in0=view3(ex[:], [[1, 36], [36, 8]]),
                    in1=AP(rca.tensor, rca.offset, [rca.ap[0], [1, 36], [0, 8]]),
                    op=OP.mult)
                nc.sync.dma_start(out=dbg["d_at"][:], in_=at[:])

            avT = []
            for fc in range(2):
                ae = psum.tile([128, 288], F32, space="PSUM", tag="psA", bufs=3, name="aeP")
                nc.tensor.matmul(out=ae[:], lhsT=wsl(f"e{fc}", rows=8, width=128),
                                 rhs=ex[:], start=True, stop=True)
                pr = pool.tile([128, 288], BF16, tag=f"pr{fc}", name=f"pr{fc}")
                nc.vector.tensor_tensor(out=pr[:], in0=vS[fc][:], in1=ae[:],
                                        op=OP.mult)
                avu = pool.tile([128, 36], F32, tag=f"avu{fc}", name=f"avu{fc}")
                nc.vector.reduce_sum(out=avu[:], in_=view3(pr[:], [[1, 36], [36, 8]]),
                                     axis=mybir.AxisListType.X)
                re = psum.tile([128, 288], F32, space="PSUM", tag="psA", bufs=3, name="reP")
                nc.tensor.matmul(out=re[:, :36], lhsT=wsl(f"e{fc}", rows=8, width=128),
                                 rhs=rc[:], start=True, stop=True)
                av = pool.tile([128, 36], BF16, tag=f"avT{fc}", name=f"avT{fc}")
                nc.vector.tensor_tensor(out=av[:], in0=avu[:], in1=re[:, :36],
                                        op=OP.mult)
                avT.append(av)
            if debug:
                t = pool.tile([128, 36], F32)
                nc.vector.tensor_copy(out=t[:], in_=avT[0][:])
                nc.sync.dma_start(out=dbg["d_av0"][:], in_=t[:])

            # ---- 15. out = attn_out @ W_out + b_out + identity (single DMA)
            oT = pool.tile([128, 72], F32)
            for mc in range(2):
                p = psum.tile([128, 288], F32, space="PSUM", tag="psA", bufs=3, name="oP")
                for kc in range(2):
                    nc.tensor.matmul(
                        out=p[:, :36], lhsT=wsl("wout", off=(kc * 2 + mc) * 128, width=128),
                        rhs=avT[kc][:], start=(kc == 0), stop=(kc == 1))
                nc.scalar.activation(out=oT[:, mc * 36:(mc + 1) * 36],
                                     in_=p[:, :36], func=AF.Identity,
                                     bias=xfsl("bout", off=mc, width=1))
                nc.vector.tensor_tensor(out=oT[:, mc * 36:(mc + 1) * 36],
                                        in0=oT[:, mc * 36:(mc + 1) * 36],
                                        in1=deT[mc], op=OP.add)
            ota = oT[:]
            oda = out[:]
            nc.sync.dma_start(
                out=AP(oda.tensor, oda.offset, [[36, 128], [128 * 36, 2], [1, 36]]),
                in_=AP(ota.tensor, ota.offset, [[72, 128], [36, 2], [1, 36]]))

    return nc


# ------------------------------------------------------------------- driver

def make_in_maps(dec_embed, bev_feat, query_scale, ref_points, weights):
    wbs = pack_wblobs(weights)
    w1 = wbs["W1"].astype(BFNP)
    w2 = wbs["W2"].astype(BFNP)
    in_maps = []
    bev16 = {}
    for c in range(8):
        b, kh = c // 2, c % 2
        if b not in bev16:
            hwc = np.ascontiguousarray(
                bev_feat[b].transpose(1, 2, 0).reshape(H * W, 256)).astype(BFNP)
            bv = np.zeros((H * W, 512), BFNP)
            bv[:, 0:256] = hwc
            bv[:(H - 1) * W, 256:512] = hwc[W:]
            bev16[b] = np.ascontiguousarray(bv)
        w0, xf, xr = pack_xblobs(weights, dec_embed, query_scale, ref_points,
                                 b, 3 * kh, wbs["W0"])
        in_maps.append({"bev": bev16[b], "wblW0": w0, "wblW1": w1,
                        "wblW2": w2, "xfl": xf, "xrl": xr})
    return in_maps


def assemble_output(results):
    out = np.zeros((K, B, T, DIM), np.float32)
    for c in range(8):
        b, kh = c // 2, c % 2
        oc = results[c]["out"]                     # (256, 36)
        out[3 * kh:3 * kh + 3, b] = oc.T.reshape(3, T, DIM)
    return out


_WNAMES = ["W_con_q", "b_con_q", "W_con_k", "W_v", "Wq1", "bq1", "Wq2", "bq2",
           "Wk1", "bk1", "Wk2", "bk2", "Wo1", "bo1", "Wo2", "bo2",
           "W_out", "b_out"]


def kernel(**inputs):
    from concourse.bass_utils import run_bass_kernel_spmd
    dec_embed = np.asarray(inputs["dec_embed"], np.float32)
    bev_feat = np.asarray(inputs["bev_feat"], np.float32)
    query_scale = np.asarray(inputs["query_scale"], np.float32)
    ref_points = np.asarray(inputs["ref_points"], np.float32)
    weights = {n: np.asarray(inputs[n], np.float32) for n in _WNAMES}

    nc = build_nc(sim_mode=False, debug=False)
    split_multiwaits(nc)
    in_maps = make_in_maps(dec_embed, bev_feat, query_scale, ref_points, weights)
    res = run_bass_kernel_spmd(nc, in_maps, list(range(8)))
    return assemble_output(res.results)
